# revision 4
# baseline (speedup 1.0000x reference)
"""CNNTransMIL kernel for 8 TRN2 NeuronCores.

Device (Bass, SPMD over 8 cores, segment-sharded): the dominant patch-embed
matmul [512 segs x 16384] @ [16384 x 1536] fused with fc1 -> relu, per core.
Host: the small transformer tail (2 Nystrom layers on 2x2048x1024, PPEG, head)
in numpy fp32.
"""

import numpy as np
import ml_dtypes

B, NSEG, L, INC = 2, 2047, 4096, 4
EMBED = 1536
DRUG = 512
KMER = 512
DIM = 1024
HEADS = 8
LM = 512  # landmarks
RES_K = 33
NCORES = 8
SEGS_PER_CORE = 512  # 2048 padded segs per batch / 4 cores per batch
K_FULL = L * INC  # 16384

bf16 = ml_dtypes.bfloat16

_COMPILED = {}


def _build_nc():
    import concourse.bacc as bacc
    import concourse.tile as tile
    import concourse.mybir as mybir

    nc = bacc.Bacc("TRN2", target_bir_lowering=False, debug=False,
                   num_devices=NCORES)
    xt_d = nc.dram_tensor("xt", [K_FULL, SEGS_PER_CORE], mybir.dt.bfloat16,
                          kind="ExternalInput")
    pw_d = nc.dram_tensor("pw", [K_FULL, EMBED], mybir.dt.bfloat16,
                          kind="ExternalInput")
    w1_d = nc.dram_tensor("w1t", [EMBED, DIM], mybir.dt.bfloat16,
                          kind="ExternalInput")
    cv_d = nc.dram_tensor("cvec", [128, DIM // 128], mybir.dt.float32,
                          kind="ExternalInput")
    ht_d = nc.dram_tensor("hT", [DIM, SEGS_PER_CORE], mybir.dt.float32,
                          kind="ExternalOutput")

    NE = EMBED // 128      # 12 emb tiles
    ND = DIM // 128        # 8 dim tiles
    NPASS = 2              # split K into halves so xt fits in SBUF
    KH = K_FULL // NPASS   # 8192 rows per pass
    NKT = KH // 128        # 64 k-tiles per pass

    with tile.TileContext(nc) as tc:
        with (
            tc.tile_pool(name="xt", bufs=1) as xt_pool,
            tc.tile_pool(name="pw", bufs=3) as pw_pool,
            tc.tile_pool(name="acc", bufs=1) as acc_pool,
            tc.tile_pool(name="w1", bufs=1) as w1_pool,
            tc.tile_pool(name="misc", bufs=1) as misc_pool,
            tc.tile_pool(name="out", bufs=2) as out_pool,
            tc.tile_pool(name="ps", bufs=4, space="PSUM") as ps_pool,
        ):
            # fp32 accumulators for xe^T: 12 tiles of [128, 512]
            xe_acc = [acc_pool.tile([128, SEGS_PER_CORE], mybir.dt.float32,
                                    name=f"xe{e}", tag=f"xe{e}") for e in range(NE)]
            cvec = misc_pool.tile([128, ND], mybir.dt.float32, tag="cv")
            nc.sync.dma_start(cvec[:], cv_d[:, :])

            for p in range(NPASS):
                # xt half: [128, NKT, 512] bf16  (8 MB)
                xt_sb = xt_pool.tile([128, NKT, SEGS_PER_CORE],
                                     mybir.dt.bfloat16, tag="xt")
                src = xt_d[p * KH:(p + 1) * KH, :].rearrange(
                    "(a q) s -> q a s", q=128)
                nc.sync.dma_start(xt_sb[:], src)
                for e in range(NE):
                    # pw slice for this (pass, e): [128, NKT, 128] bf16 (2 MB)
                    pw_sb = pw_pool.tile([128, NKT, 128], mybir.dt.bfloat16,
                                         tag="pw")
                    psrc = pw_d[p * KH:(p + 1) * KH,
                                e * 128:(e + 1) * 128].rearrange(
                        "(a q) m -> q a m", q=128)
                    nc.sync.dma_start(pw_sb[:], psrc)
                    ps = ps_pool.tile([128, SEGS_PER_CORE], mybir.dt.float32,
                                      tag="ps")
                    for k in range(NKT):
                        nc.tensor.matmul(ps[:], pw_sb[:, k, :],
                                         xt_sb[:, k, :],
                                         start=(k == 0), stop=(k == NKT - 1))
                    if p == 0:
                        nc.scalar.activation(xe_acc[e][:], ps[:],
                                             mybir.ActivationFunctionType.Copy)
                    else:
                        nc.vector.tensor_add(xe_acc[e][:], xe_acc[e][:],
                                             ps[:])

            # cast xe^T to bf16 for the fc1 matmul
            xe_bf = [acc_pool.tile([128, SEGS_PER_CORE], mybir.dt.bfloat16,
                                   name=f"xb{e}", tag=f"xb{e}") for e in range(NE)]
            for e in range(NE):
                nc.vector.tensor_copy(xe_bf[e][:], xe_acc[e][:])

            # fc1 weights resident: 12 tiles [128, 1024] bf16 (3 MB)
            w1_sb = w1_pool.tile([128, NE, DIM], mybir.dt.bfloat16, tag="w1")
            nc.sync.dma_start(
                w1_sb[:], w1_d[:, :].rearrange("(a q) m -> q a m", q=128))

            for d in range(ND):
                ps = ps_pool.tile([128, SEGS_PER_CORE], mybir.dt.float32,
                                  tag="ps2")
                for e in range(NE):
                    nc.tensor.matmul(ps[:],
                                     w1_sb[:, e, d * 128:(d + 1) * 128],
                                     xe_bf[e][:],
                                     start=(e == 0), stop=(e == NE - 1))
                hrow = out_pool.tile([128, SEGS_PER_CORE], mybir.dt.float32,
                                     tag="h")
                nc.scalar.activation(hrow[:], ps[:],
                                     mybir.ActivationFunctionType.Relu,
                                     bias=cvec[:, d:d + 1], scale=1.0)
                nc.sync.dma_start(ht_d[d * 128:(d + 1) * 128, :], hrow[:])
    nc.compile()
    return nc


# ---------------- host-side transformer tail (numpy fp32) ----------------

def _layer_norm(x, g, b, eps=1e-5):
    mu = x.mean(-1, keepdims=True)
    var = ((x - mu) ** 2).mean(-1, keepdims=True)
    return (x - mu) / np.sqrt(var + eps) * g + b


def _softmax(x):
    m = x.max(-1, keepdims=True)
    e = np.exp(x - m)
    return e / e.sum(-1, keepdims=True)


def _pinv(x, iters=6):
    ax = np.abs(x)
    scale = ax.sum(-1).max() * ax.sum(-2).max()
    z = np.swapaxes(x, -1, -2) / scale
    I = np.eye(x.shape[-1], dtype=x.dtype)
    for _ in range(iters):
        xz = x @ z
        z = 0.25 * z @ (13 * I - xz @ (15 * I - xz @ (7 * I - xz)))
    return z


def _nystrom(x, qkv_w, out_w, out_b, res_w):
    b, n, _ = x.shape  # n == 2048, no padding needed (2048 % 512 == 0)
    qkv = x @ qkv_w.T
    q, k, v = np.split(qkv, 3, axis=-1)
    dh = DIM // HEADS
    sh = lambda t: t.reshape(b, n, HEADS, dh).transpose(0, 2, 1, 3)
    q = sh(q) * (dh ** -0.5)
    k = sh(k)
    v = sh(v)
    lgrp = n // LM
    q_l = q.reshape(b, HEADS, LM, lgrp, dh).mean(3)
    k_l = k.reshape(b, HEADS, LM, lgrp, dh).mean(3)
    kt = np.swapaxes(k_l, -1, -2)
    a1 = _softmax(q @ kt)
    a2 = _softmax(q_l @ kt)
    a3 = _softmax(q_l @ np.swapaxes(k, -1, -2))
    out = (a1 @ _pinv(a2)) @ (a3 @ v)
    # depthwise residual conv over sequence dim, one 33-tap filter per head
    pad = RES_K // 2
    vp = np.pad(v, ((0, 0), (0, 0), (pad, pad), (0, 0)))
    res = np.zeros_like(v)
    for t in range(RES_K):
        res += vp[:, :, t:t + n, :] * res_w[:, 0, t, 0][None, :, None, None]
    out = (out + res).transpose(0, 2, 1, 3).reshape(b, n, DIM)
    return out @ out_w.T + out_b


def _ppeg(x, w7, b7, w5, b5, w3, b3):
    cls_tok = x[:, :1]
    f = np.swapaxes(x[:, 1:], 1, 2)  # [B, C, N']
    npr = f.shape[2]
    for w, bb in ((w7, b7), (w5, b5), (w3, b3)):
        ksz = w.shape[-1]
        pad = ksz // 2
        fp = np.pad(f, ((0, 0), (0, 0), (pad, pad)))
        conv = np.zeros_like(f)
        for t in range(ksz):
            conv += fp[:, :, t:t + npr] * w[:, 0, t][None, :, None]
        f = f + conv + bb[None, :, None]
    return np.concatenate([cls_tok, np.swapaxes(f, 1, 2)], axis=1)


def kernel(x, drug, H_kmer, patch_w, patch_b, kmer_g, kmer_b, fc1_w, fc1_b,
           cls_token, ln1_g, ln1_b, qkv1_w, out1_w, out1_b, res1_w,
           ppeg_w7, ppeg_b7, ppeg_w5, ppeg_b5, ppeg_w3, ppeg_b3,
           ln2_g, ln2_b, qkv2_w, out2_w, out2_b, res2_w,
           normf_g, normf_b, fc2_w, fc2_b):
    from concourse.bass_utils import run_bass_kernel_spmd

    x = np.asarray(x, dtype=np.float32)
    # ---- host prep ----
    # flatten (s, c) -> k = s*4 + c; pw[k, o] = patch_w[o, c, s]
    pw = np.ascontiguousarray(
        np.transpose(np.asarray(patch_w, np.float32), (2, 1, 0))
        .reshape(K_FULL, EMBED)).astype(bf16)
    W1a = np.asarray(fc1_w, np.float32)[:, :EMBED]           # [1024, 1536]
    Wdr = np.asarray(fc1_w, np.float32)[:, EMBED:EMBED + DRUG]
    Wkm = np.asarray(fc1_w, np.float32)[:, EMBED + DRUG:]
    w1t = np.ascontiguousarray(W1a.T).astype(bf16)           # [1536, 1024]
    hk = _layer_norm(np.asarray(H_kmer, np.float32),
                     np.asarray(kmer_g, np.float32),
                     np.asarray(kmer_b, np.float32))
    # per-batch fused bias: drug/kmer/fc1_b plus patch_b folded through W1a
    cvecs = []
    for b in range(B):
        c = (Wdr @ np.asarray(drug, np.float32)[b, 0]
             + Wkm @ hk[b]
             + np.asarray(fc1_b, np.float32)
             + W1a @ np.asarray(patch_b, np.float32))
        cvecs.append(np.ascontiguousarray(
            c.reshape(DIM // 128, 128).T.astype(np.float32)))

    xf = x.reshape(B, NSEG, K_FULL)
    xpad = np.zeros((B, 4 * SEGS_PER_CORE, K_FULL), np.float32)
    xpad[:, :NSEG] = xf
    in_maps = []
    for core in range(NCORES):
        b, j = divmod(core, 4)
        slab = xpad[b, j * SEGS_PER_CORE:(j + 1) * SEGS_PER_CORE]
        in_maps.append({
            "xt": np.ascontiguousarray(slab.T).astype(bf16),
            "pw": pw,
            "w1t": w1t,
            "cvec": cvecs[b],
        })

    key = "nc"
    if key not in _COMPILED:
        _COMPILED[key] = _build_nc()
    nc = _COMPILED[key]

    res = run_bass_kernel_spmd(nc, in_maps, core_ids=list(range(NCORES)),
                               trace=False)

    h = np.empty((B, NSEG, DIM), np.float32)
    for core in range(NCORES):
        b, j = divmod(core, 4)
        lo = j * SEGS_PER_CORE
        hi = min(lo + SEGS_PER_CORE, NSEG)
        h[b, lo:hi] = res.results[core]["hT"][:, :hi - lo].T

    # ---- host transformer tail ----
    cls_b = np.broadcast_to(np.asarray(cls_token, np.float32), (B, 1, DIM))
    h = np.concatenate([cls_b, h], axis=1)  # [B, 2048, 1024]
    h = h + _nystrom(_layer_norm(h, np.asarray(ln1_g, np.float32),
                                 np.asarray(ln1_b, np.float32)),
                     np.asarray(qkv1_w, np.float32),
                     np.asarray(out1_w, np.float32),
                     np.asarray(out1_b, np.float32),
                     np.asarray(res1_w, np.float32))
    h = _ppeg(h, np.asarray(ppeg_w7, np.float32), np.asarray(ppeg_b7, np.float32),
              np.asarray(ppeg_w5, np.float32), np.asarray(ppeg_b5, np.float32),
              np.asarray(ppeg_w3, np.float32), np.asarray(ppeg_b3, np.float32))
    h = h + _nystrom(_layer_norm(h, np.asarray(ln2_g, np.float32),
                                 np.asarray(ln2_b, np.float32)),
                     np.asarray(qkv2_w, np.float32),
                     np.asarray(out2_w, np.float32),
                     np.asarray(out2_b, np.float32),
                     np.asarray(res2_w, np.float32))
    h = _layer_norm(h, np.asarray(normf_g, np.float32),
                    np.asarray(normf_b, np.float32))[:, 0]
    return (h @ np.asarray(fc2_w, np.float32).T
            + np.asarray(fc2_b, np.float32)).astype(np.float32)
